# revision 68
# baseline (speedup 1.0000x reference)
"""Trainium2 Bass kernel for a dense causal-attention transformer block.

Full computation (matches the fp32 jax reference):
    qkv = x @ w_qkv ; split q,k,v ; heads 16x64 ; q *= 64**-0.5
    rotary (first 32 dims) applied to q, k AND v
    causal softmax attention ; merge heads ; @ w_out ; LayerNorm(g)

Sharding over 8 NeuronCores: core = b*4 + s*2 + h
    b: batch (2)   s: sequence half (even/odd 128-row blocks)   h: head half
Each core ships x with columns in ITS key order P = [own-q-desc ++
partner-q-desc] so (a) the Q projection is just the first 1024 columns of
xT (no duplicate load) and (b) every key block's visible q-set is a
contiguous prefix, uniform across cores; the residual s-asymmetry of the
odd/even split lives in a tiny [128,256] diagonal-mask input.
Cross-core exchange: four 2-core AllGathers (0.25 MB bf16 each) of the
per-head-pair attention outputs, issued as each head pair finishes; the
gathered tiles are copied to SBUF only after the attention loop so a
skew-delayed collective can never block the DMA queue mid-attention.
The output projection is COLUMN-split across the pair (each core holds its
512 columns of w_out), and LayerNorm statistics are reconstructed from an
8 KB AllReduce of per-row partial sums (sum, sum-of-squares).

All matmuls in bf16 with fp32 PSUM accumulation; softmax in fp32 without
max-subtraction (scores are O(5), exp is safe); mask=ones per the input spec.
"""

import numpy as np
import ml_dtypes

import concourse.bass as bass
import concourse.mybir as mybir
import concourse.tile as tile
from concourse import bacc
from concourse.bass_utils import run_bass_kernel_spmd

dt = mybir.dt
F = mybir.ActivationFunctionType
OP = mybir.AluOpType
bf16 = ml_dtypes.bfloat16

B, N, DIM = 2, 2048, 1024
HEADS, DH, ROT = 16, 64, 32
HALF = ROT // 2  # 16
SCALE = DH ** -0.5
NCORES = 8
NB = N // 128            # 16 global q/k blocks
BLOCKS = {0: [14, 12, 10, 8, 6, 4, 2, 0], 1: [15, 13, 11, 9, 7, 5, 3, 1]}
PAIRS = [[0, 1], [2, 3], [4, 5], [6, 7]]
# attention iteration order per head pair: all OWN key blocks first (their
# K/V projections only need the first half of xT, so attention starts while
# the second half still streams in), then the partner blocks
ITER = [(j, 0) for j in range(7, -1, -1)] + [(j, 1) for j in range(7, -1, -1)]
HI_STOP = ITER.index((4, 1))   # last iteration touching psum cols 512:1024

TRACE = False
LAST_EXEC_NS = None
LAST_RESULTS = None

_CACHE = {}


def _build_program():
    nc = bacc.Bacc(trn_type="TRN2", target_bir_lowering=False, debug=False,
                   num_devices=NCORES)

    # activation/weight tensors are stored TILE-CONTIGUOUS on the host
    # (each tile one contiguous block) so load DMAs use wide per-partition
    # rows (2-4 KB descriptors instead of 1 KB)
    d_xT = nc.dram_tensor("xT", [16 * 128, 1024], dt.bfloat16, kind="ExternalInput")
    d_wkv = nc.dram_tensor("wkv", [8 * 128, 1024], dt.bfloat16, kind="ExternalInput")
    d_wq = nc.dram_tensor("wq", [8 * 128, 512], dt.bfloat16, kind="ExternalInput")
    d_wout = nc.dram_tensor("wout", [8 * 128, 512], dt.bfloat16, kind="ExternalInput")
    # rope tables: both 32-row head slots hold identical data; ship once
    d_cosK = nc.dram_tensor("cosK", [32, N], dt.bfloat16, kind="ExternalInput")
    d_sinK = nc.dram_tensor("sinK", [32, N], dt.bfloat16, kind="ExternalInput")
    d_cosVs = nc.dram_tensor("cosVs", [128, 512], dt.bfloat16, kind="ExternalInput")
    d_sinVs = nc.dram_tensor("sinVs", [128, 512], dt.bfloat16, kind="ExternalInput")
    d_maskD = nc.dram_tensor("maskD", [128, 256], dt.bfloat16, kind="ExternalInput")
    d_g = nc.dram_tensor("gw", [1, 512], dt.bfloat16, kind="ExternalInput")
    d_out = nc.dram_tensor("out", [1024, 512], dt.bfloat16, kind="ExternalOutput")

    with tile.TileContext(nc) as tc:
        with (
            tc.tile_pool(name="cst", bufs=1) as cst,
            tc.tile_pool(name="dram", bufs=1, space="DRAM") as dpool,
        ):
            # ---- persistent SBUF tiles
            kT = cst.tile([128, 4 * N], dt.bfloat16, tag="kT")       # [2head-dims, key slots]
            vA = cst.tile([128, 16 * 520], dt.bfloat16, tag="vA")    # rows; per chunk 8x(64 dims + 1 one)
            qT = cst.tile([128, 4 * 1024], dt.bfloat16, tag="qT")
            outT = cst.tile([128, 4 * 1024], dt.bfloat16, tag="outT")
            cosK = cst.tile([128, N], dt.bfloat16, tag="cosK")
            sinK = cst.tile([128, N], dt.bfloat16, tag="sinK")
            maskD = cst.tile([128, 256], dt.bfloat16, tag="maskD")
            g_bc = cst.tile([128, 512], dt.bfloat16, tag="g_bc")
            g_row = cst.tile([1, 512], dt.bfloat16, tag="g_row")
            eps_t = cst.tile([128, 1], dt.float32, tag="eps_t")
            wout_t = [cst.tile([128, 512], dt.bfloat16, name=f"wout{k}", tag=f"wout{k}") for k in range(8)]
            oT = [cst.tile([128, 1024], dt.bfloat16, name=f"oT{k}", tag=f"oT{k}") for k in range(8)]

            agin4 = [dpool.tile([128, 1024], dt.bfloat16, name=f"agin{t}") for t in range(4)]
            agout4 = [dpool.tile([256, 1024], dt.bfloat16, name=f"agout{t}") for t in range(4)]
            statsD = [dpool.tile([128, 8], dt.float32, name=f"statsD{i}") for i in range(2)]
            statsR = [dpool.tile([256, 8], dt.float32, name=f"statsR{i}") for i in range(2)]

            # alternate input loads across the two HW DMA queues (SP + ACT):
            # the Sync sequencer takes ~600ns per DMA issue, so a single
            # queue serializes the load phase
            def load(i, dst, src):
                (nc.sync if i % 2 == 0 else nc.scalar).dma_start(dst, src)

            # ================= projections =================
            with (
                tc.tile_pool(name="xw", bufs=1) as xw,
                tc.tile_pool(name="rotp", bufs=1) as rotp,
                tc.tile_pool(name="simp", bufs=2, space="PSUM") as simp,
                tc.tile_pool(name="outp", bufs=1, space="PSUM") as outp,
            ):
                xT_t = [xw.tile([128, N], dt.bfloat16, name=f"xT{k}", tag=f"xT{k}") for k in range(8)]
                wkv_t = [xw.tile([128, 1024], dt.bfloat16, name=f"wkv{k}", tag=f"wkv{k}") for k in range(8)]
                wq_t = [xw.tile([128, 512], dt.bfloat16, name=f"wq{k}", tag=f"wq{k}") for k in range(8)]

                # --- input DMAs in consumption order
                def tload(i, dst, dten, t):
                    load(i, dst, dten.ap()[t * 128:(t + 1) * 128, :])

                for k in range(8):
                    tload(k, wkv_t[k][:, 0:1024], d_wkv, k)
                for k in range(8):
                    tload(k, xT_t[k][:, 0:1024], d_xT, k)
                for k in range(8):
                    tload(k, wq_t[k][:], d_wq, k)
                for lo in (0, 64):
                    load(0, cosK[lo:lo + 32, :], d_cosK.ap()[:])
                    load(1, sinK[lo:lo + 32, :], d_sinK.ap()[:])
                vrot = tc.alloc_tile_pool(name="vrot", bufs=1)
                cosV = vrot.tile([128, 4096], dt.bfloat16, tag="cosV")
                sinV = vrot.tile([128, 4096], dt.bfloat16, tag="sinV")
                cosVs = vrot.tile([128, 512], dt.bfloat16, tag="cosVs")
                sinVs = vrot.tile([128, 512], dt.bfloat16, tag="sinVs")
                load(0, cosVs[:], d_cosVs.ap()[:])
                load(1, sinVs[:], d_sinVs.ap()[:])
                cV4 = cosV[:, 0:4096].rearrange("p (c h e) -> p c h e", c=16, h=8)
                sV4 = sinV[:, 0:4096].rearrange("p (c h e) -> p c h e", c=16, h=8)
                # broadcast the per-position rotary tables across the 8 head
                # slots on the ACT engine (a strided DMA would shatter into
                # 64-byte descriptors)
                for h8 in range(8):
                    nc.scalar.copy(cV4[:, :, h8, :],
                                   cosVs[:, :].rearrange("p (c e) -> p c e", c=16))
                    nc.scalar.copy(sV4[:, :, h8, :],
                                   sinVs[:, :].rearrange("p (c e) -> p c e", c=16))
                load(0, maskD[:], d_maskD.ap()[:])
                load(1, g_row[:], d_g.ap()[:])
                for k in range(8):
                    tload(k, xT_t[k][:, 1024:2048], d_xT, 8 + k)
                for k in range(8):
                    tload(k, wout_t[k][:], d_wout, k)
                nc.gpsimd.partition_broadcast(g_bc[:], g_row[:])
                nc.vector.memset(eps_t[:], 1e-5)

                # ones column of the [v | 1] PV weights (col 64 of each 65-slot)
                nc.vector.memset(
                    vA[:, 0:16 * 520].rearrange("p (c h e) -> p c h e", c=16, h=8)[:, :, :, 64:65],
                    1.0,
                )

                # rotary for a column window of a head-pair tile, in place:
                # swap the 16-row halves via SBUF->SBUF DMA, then
                # t' = t*cos + swapped*sin_signed on the {0:32},{64:96} rows.
                # The DVE multiplies are split into 512-column chunks so the
                # burst can be spread across attention iterations instead of
                # blocking the mask chain.
                def rot_swap(t, t4, width, c0, cw, tag):
                    c = slice(t4 * width + c0, t4 * width + c0 + cw)
                    tmp = rotp.tile([128, cw], dt.bfloat16, tag=tag)
                    for lo in (0, 64):
                        nc.sync.dma_start(tmp[lo:lo + 16, :], t[lo + 16:lo + 32, c])
                        nc.sync.dma_start(tmp[lo + 16:lo + 32, :], t[lo:lo + 16, c])
                    return tmp

                def rot_mul(t, t4, width, tmp, c0, d0, dw):
                    cc = slice(t4 * width + c0 + d0, t4 * width + c0 + d0 + dw)
                    cl = slice(c0 + d0, c0 + d0 + dw)  # cosK/sinK columns
                    tl = slice(d0, d0 + dw)
                    for lo in (0, 64):
                        sl = slice(lo, lo + 32)
                        nc.vector.tensor_mul(tmp[sl, tl], tmp[sl, tl], sinK[sl, cl])
                        nc.vector.tensor_mul(t[sl, cc], t[sl, cc], cosK[sl, cl])
                        nc.vector.tensor_add(t[sl, cc], t[sl, cc], tmp[sl, tl])

                def make_rot_own(m4):
                    # own-half kT (cols 0:1024 of the t4 tile) + all of qT:
                    # a swap thunk and four 512-col multiply thunks
                    st = {}

                    def do_swap():
                        st['ko'] = rot_swap(kT, m4, N, 0, 1024, "rko")
                        st['q'] = rot_swap(qT, m4, 1024, 0, 1024, "rq")
                    # ordered by first use in the next phase: all of qT at
                    # it=0, kT cols 512:1024 (slots 7..4) before 0:512
                    muls = [lambda: rot_mul(qT, m4, 1024, st['q'], 0, 0, 512),
                            lambda: rot_mul(qT, m4, 1024, st['q'], 0, 512, 512),
                            lambda: rot_mul(kT, m4, N, st['ko'], 0, 512, 512),
                            lambda: rot_mul(kT, m4, N, st['ko'], 0, 0, 512)]
                    return do_swap, muls

                def make_rot_part(m4):
                    st = {}

                    def do_swap():
                        st['kp'] = rot_swap(kT, m4, N, 1024, 1024, "rkp")
                    muls = [lambda: rot_mul(kT, m4, N, st['kp'], 1024, 0, 512),
                            lambda: rot_mul(kT, m4, N, st['kp'], 1024, 512, 512)]
                    return do_swap, muls

                # one kT or qT projection psum-group; shares the simp PSUM
                # ring with attention so groups for head pair m4 can be
                # interleaved into head pair m4-1's attention, keeping the
                # PE dense
                def emit_kq_group(m4, gi, on_act=False):
                    cp = nc.scalar.copy if on_act else nc.vector.tensor_copy
                    ps = simp.tile([128, 1024], dt.float32, tag="s_ps", name=f"pjg{m4}_{gi}")
                    if gi < 4:
                        for k in range(8):
                            nc.tensor.matmul(ps[:, 0:512], wkv_t[k][:, m4 * 128:(m4 + 1) * 128],
                                             xT_t[k][:, gi * 512:(gi + 1) * 512],
                                             start=(k == 0), stop=(k == 7))
                        cp(kT[:, m4 * N + gi * 512: m4 * N + (gi + 1) * 512], ps[:, 0:512])
                    else:
                        nn = gi - 4
                        for k in range(8):
                            nc.tensor.matmul(ps[:, 0:512], wq_t[k][:, m4 * 128:(m4 + 1) * 128],
                                             xT_t[k][:, nn * 512:(nn + 1) * 512],
                                             start=(k == 0), stop=(k == 7))
                        cp(qT[:, m4 * 1024 + nn * 512: m4 * 1024 + (nn + 1) * 512], ps[:, 0:512])


                # v chunk r: natural rows x (8 heads x 64), strided into
                # 65-slots; copies on ACT (DVE is loaded with rotary)
                tmpV = vrot.tile([128, 4 * 256], dt.bfloat16, tag="tmpV")
                v4 = vA[:, 0:16 * 520].rearrange("p (c h e) -> p c h e", c=16, h=8)
                t4v = tmpV[:, 0:4 * 256].rearrange("p (c h e) -> p c h e", c=4, h=8)

                def emit_v_chunk(r, rot):
                    ps = simp.tile([128, 1024], dt.float32, tag="s_ps", name=f"vps{r}")
                    for k in range(8):
                        nc.tensor.matmul(ps[:, 0:512], xT_t[k][:, r * 128:(r + 1) * 128],
                                         wkv_t[k][:, 512:1024],
                                         start=(k == 0), stop=(k == 7))
                    nc.scalar.copy(
                        vA[:, r * 520: r * 520 + 520].rearrange("p (h e) -> p h e", h=8)[:, :, 0:64],
                        ps[:, 0:512].rearrange("p (h e) -> p h e", h=8),
                    )
                    if rot is not None:
                        # tmp[a] = v[b]*sinS[a]; tmp[b] = v[a]*sinS[b]
                        sl = rot
                        nw = sl.stop - sl.start
                        tv = t4v[:, 0:nw]
                        nc.vector.tensor_mul(tv[:, :, :, 0:16], v4[:, sl, :, 16:32], sV4[:, sl, :, 0:16])
                        nc.vector.tensor_mul(tv[:, :, :, 16:32], v4[:, sl, :, 0:16], sV4[:, sl, :, 16:32])
                        nc.vector.tensor_mul(v4[:, sl, :, 0:32], v4[:, sl, :, 0:32], cV4[:, sl, :, 0:32])
                        nc.vector.tensor_add(v4[:, sl, :, 0:32], v4[:, sl, :, 0:32], tv[:, :, :, 0:32])

                # projection order puts everything that only needs the first
                # half of xT (key slots 0..7 = the core's own q columns)
                # first: own-K, Q, own rotary, own-V (descending, the order
                # attention consumes them).  Partner-half projections (gi
                # 2,3), their rotary, and v chunks 15..8 stream into the
                # first attention phase's slots.
                for gi in (0, 1, 4, 5):
                    emit_kq_group(0, gi, on_act=True)
                swap0, muls0 = make_rot_own(0)
                swap0()
                for th in muls0:
                    th()
                for r in (7, 6):
                    emit_v_chunk(r, slice(r, r + 1))

                # ================= attention =================
                # key slot for (j, par): own prefix-j at slot j, partner at 8+j
                att = tc.alloc_tile_pool(name="att", bufs=7)
                nrm = tc.alloc_tile_pool(name="nrm", bufs=2)
                nrm1 = tc.alloc_tile_pool(name="nrm1", bufs=2)

                # normalize: rows 0..63 = sum(exp*v), row 64 = sum(exp).
                # Copy PSUM out fast (so the next head pair's PV — or the
                # out-projection, for the last pair — can reuse the
                # accumulator banks), then 1/s on DVE with the sums row
                # spread across 128 partitions via a DRAM bounce
                # (single-partition DVE ops cost ~3.3us; spread ones are
                # ~0.1us); partition-broadcast on GpSimd.  The final
                # per-head 1/s multiplies (and the AllGather behind them)
                # are returned as deferred thunks: running them inside the
                # NEXT phase keeps the DVE queue free for its mask chain.
                def norm(t4, o_2, pool=None, pool1=None, sfx="", eng=None):
                    pool = pool or nrm
                    pool1 = pool1 or nrm1
                    eng = eng or nc.sync
                    osb = []
                    for hsi in range(2):
                        ot = pool.tile([65, 1024], dt.bfloat16, name=f"osb{hsi}{sfx}",
                                       tag=f"osb{hsi}{sfx}")
                        nc.vector.tensor_copy(ot[:], o_2[hsi][:])
                        osb.append(ot)
                    sh = pool1.tile([128, 16], dt.bfloat16, tag="sh" + sfx)
                    shf = pool1.tile([128, 16], dt.float32, tag="shf" + sfx)
                    shr = pool1.tile([128, 16], dt.bfloat16, tag="shr" + sfx)
                    for hsi in range(2):
                        dsum = dpool.tile([1, 1024], dt.bfloat16, name=f"dsum{t4}_{hsi}")
                        eng.dma_start(dsum[:], osb[hsi][64:65, :])
                        eng.dma_start(
                            sh[:, hsi * 8:(hsi + 1) * 8],
                            dsum[0:1, :].rearrange("o (p c) -> (o p) c", p=128))
                    nc.vector.tensor_copy(shf[:], sh[:])
                    nc.vector.reciprocal(shf[:], shf[:])
                    nc.vector.tensor_copy(shr[:], shf[:])
                    thunks = []
                    for hsi in range(2):
                        def mul_thunk(hsi=hsi):
                            hb = hsi * 64
                            drs = dpool.tile([1, 1024], dt.bfloat16, name=f"drs{t4}_{hsi}")
                            rs2 = pool1.tile([1, 1024], dt.bfloat16, tag="rs2" + sfx)
                            bc = pool1.tile([64, 1024], dt.bfloat16, tag="bc" + sfx)
                            eng.dma_start(
                                drs[0:1, :].rearrange("o (p c) -> (o p) c", p=128),
                                shr[:, hsi * 8:(hsi + 1) * 8])
                            eng.dma_start(rs2[:], drs[:])
                            nc.gpsimd.partition_broadcast(bc[:], rs2[:])
                            nc.vector.tensor_mul(outT[hb:hb + 64, t4 * 1024:(t4 + 1) * 1024],
                                                 osb[hsi][0:64, :], bc[:])
                        thunks.append(mul_thunk)
                    thunks.append(lambda: ship(t4))
                    return thunks

                def ship(t4, eng=None):
                    # ship this head-pair's output to the partner; the
                    # transfer hides under the remaining attention
                    (eng or nc.sync).dma_start(agin4[t4][:], outT[:, t4 * 1024:(t4 + 1) * 1024])
                    nc.gpsimd.collective_compute(
                        "AllGather", OP.bypass, replica_groups=PAIRS,
                        ins=[agin4[t4].opt()], outs=[agout4[t4].opt()],
                    )

                deferred = []
                for t4 in range(4):
                    # both heads of the pair interleaved; PV of iteration it
                    # is emitted after QK/exp of it+2 (two-block software
                    # pipeline) so the PE never waits on the exp chain
                    o_2 = [outp.tile([65, 1024], dt.float32, name=f"o2_{t4}_{i}", tag=f"o2_{i}")
                           for i in range(2)]

                    def emit_pv(it, hsi, w, eT, t4=t4, o_2=o_2):
                        slot = ITER[it][0] + 8 * ITER[it][1]
                        lh = t4 * 2 + hsi
                        for c0 in range(0, w, 512):
                            cw = min(512, w - c0)
                            nc.tensor.matmul(
                                o_2[hsi][:, c0:c0 + cw],
                                vA[:, slot * 520 + lh * 65: slot * 520 + lh * 65 + 65],
                                eT[:, c0:c0 + cw],
                                start=(it == 0),
                                stop=(it == (15 if c0 == 0 else HI_STOP)))

                    # per-iteration side work, scheduled so that by the time
                    # the NEXT phase starts its own-key half, that half's
                    # projections + rotary are done, and this phase's
                    # partner-key half (it>=8) has ITS dependencies done:
                    #   t4=0 only: gi 2,3 + partner rotary of pair 0, and v
                    #     chunks 15..8, early
                    #   all t4<3: next pair's 6 projection groups, own+q
                    #     rotary mid-phase, partner rotary at the end
                    #   previous phase's deferred 1/s multiplies + AllGather
                    #     at 1/3/5
                    slots = {}
                    spill = []

                    def sched(it_s, th):
                        if it_s is None:
                            spill.append(th)
                        else:
                            slots.setdefault(it_s, []).append(th)

                    for i, th in enumerate(deferred):
                        sched(1 + 2 * i, th)
                    if t4 == 0:
                        # remaining own-v feeding the A-half PVs just in
                        # time; this pair's partner projections (xT second
                        # half lands mid-phase) + partner rotary before
                        # it=8; partner v; next pair's work late
                        for i, r in enumerate((5, 4, 3, 2, 1, 0)):
                            sched(i, lambda r=r: emit_v_chunk(r, slice(r, r + 1)))
                        sched(6, lambda: emit_kq_group(0, 2))
                        swp, mulp = make_rot_part(0)
                        sched(7, lambda: emit_kq_group(0, 3))
                        sched(7, swp)
                        sched(7, mulp[0])
                        sched(7, mulp[1])
                        for i, r in enumerate(range(15, 7, -1)):
                            sched(8 + i, lambda r=r: emit_v_chunk(r, slice(r, r + 1)))
                        kq_sl = (9, 10, 11, 12, 13, 14)
                        own_sl = (14, 14, 15, 15, None)
                        par_sl = (15, None, None)
                    else:
                        kq_sl = (2, 3, 4, 5, 6, 7)
                        own_sl = (6, 7, 8, 9, 10)
                        par_sl = (13, 14, 15)
                    if t4 < 3:
                        m4 = t4 + 1
                        for it_s, gi in zip(kq_sl, (0, 1, 4, 5, 2, 3)):
                            sched(it_s, lambda gi=gi, m4=m4: emit_kq_group(m4, gi))
                        swo, mulo = make_rot_own(m4)
                        sched(own_sl[0], swo)
                        for it_s, th in zip(own_sl[1:], mulo):
                            sched(it_s, th)
                        swp2, mulp2 = make_rot_part(m4)
                        sched(par_sl[0], swp2)
                        sched(par_sl[1], mulp2[0])
                        sched(par_sl[2], mulp2[1])

                    pend = []
                    for it, (j, par) in enumerate(ITER):
                        slot = j + 8 * par
                        w = (j + 1) * 128
                        sps2 = []
                        for hsi in range(2):
                            hb = hsi * 64
                            s_ps = simp.tile([128, 1024], dt.float32, tag="s_ps")
                            for c0 in range(0, w, 512):
                                cw = min(512, w - c0)
                                nc.tensor.matmul(
                                    s_ps[:, c0:c0 + cw],
                                    kT[hb:hb + 64, t4 * N + slot * 128: t4 * N + (slot + 1) * 128],
                                    qT[hb:hb + 64, t4 * 1024 + c0: t4 * 1024 + c0 + cw],
                                    start=True, stop=True)
                            sps2.append(s_ps)
                        for hsi in range(2):
                            eT = att.tile([128, 1024], dt.bfloat16, tag="eT")
                            nc.scalar.activation(eT[:, 0:w], sps2[hsi][:, 0:w], F.Exp)
                            nc.vector.tensor_mul(eT[:, w - 128:w], eT[:, w - 128:w],
                                                 maskD[:, par * 128:(par + 1) * 128])
                            pend.append((it, hsi, w, eT))
                        while len(pend) > 4:
                            emit_pv(*pend.pop(0))
                        for th in slots.get(it, []):
                            th()
                    for p in pend:
                        emit_pv(*p)

                    if t4 < 3:
                        deferred = norm(t4, o_2)
                    else:
                        # last pair: normalize immediately (its latency chain
                        # gates the final AllGather), on PERSISTENT tiles so
                        # the attention pools' release boundaries (and the
                        # out-projection's PSUM alloc behind them) never wait
                        # for it.  No DRAM bounce here: the single-partition
                        # DVE reciprocal (~3.3us) beats the bounce's ~10us
                        # of DMA latency on this exposed chain.  ship(3) is
                        # emitted AFTER the pool transition below.
                        osb3 = []
                        for hsi in range(2):
                            ot3 = cst.tile([65, 1024], dt.bfloat16, tag=f"osb{hsi}3")
                            nc.vector.tensor_copy(ot3[:], o_2[hsi][:])
                            osb3.append(ot3)
                        for hsi in range(2):
                            rec3 = cst.tile([1, 1024], dt.bfloat16, tag=f"rec{hsi}3")
                            bc3 = cst.tile([64, 1024], dt.bfloat16, tag=f"bc{hsi}3")
                            with nc.allow_low_precision(
                                    reason="1/s in bf16, same as the bounce path"):
                                nc.vector.reciprocal(rec3[:], osb3[hsi][64:65, :])
                            nc.gpsimd.partition_broadcast(bc3[:], rec3[:])
                            nc.vector.tensor_mul(
                                outT[hsi * 64:hsi * 64 + 64, 3072:4096],
                                osb3[hsi][0:64, :], bc3[:])
                    for th in spill:
                        th()
                # pull the first three pairs' gathered tiles to SBUF only
                # now: a skew-delayed AllGather can never block the DMA
                # queue behind attention-phase DMAs
                for p4 in range(3):
                    nc.sync.dma_start(oT[2 * p4][:], agout4[p4][0:128, :])
                    nc.sync.dma_start(oT[2 * p4 + 1][:], agout4[p4][128:256, :])
                nrm1.release()
                nrm.release()
                att.release()
                vrot.release()

            # ======== out projection (column-split) + layernorm ========
            # k-tiles 6,7 come from the last AllGather; accumulate them last
            # so the first k0..5 matmuls cover its latency.  LN statistics
            # (sum, sum of squares over this core's 512 columns) are summed
            # with the partner's via a tiny AllReduce.
            with (
                tc.tile_pool(name="prj", bufs=1, space="PSUM") as prj,
                tc.tile_pool(name="lnp", bufs=2) as lnp,
                tc.tile_pool(name="lns", bufs=1) as lns,
            ):
                ship(3, eng=nc.scalar)
                stats = lns.tile([128, 16], dt.float32, tag="stats")
                statsS = lns.tile([128, 16], dt.float32, tag="statsS")
                rstd4 = [lns.tile([128, 4], dt.float32, name=f"rstd4_{i}", tag=f"rstd4_{i}")
                         for i in range(2)]
                nmrs4 = [lns.tile([128, 4], dt.float32, name=f"nmrs4_{i}", tag=f"nmrs4_{i}")
                         for i in range(2)]
                nc.sync.dma_start(oT[6][:], agout4[3][0:128, :])
                nc.sync.dma_start(oT[7][:], agout4[3][128:256, :])

                def ln_send(i):
                    nc.sync.dma_start(statsD[i][:], stats[:, 8 * i:8 * i + 8])
                    nc.gpsimd.collective_compute(
                        "AllGather", OP.bypass, replica_groups=PAIRS,
                        ins=[statsD[i].opt()], outs=[statsR[i].opt()],
                    )

                def ln_math(i):
                    # batched LN math for row-blocks 4i..4i+3:
                    # rstd = 1/sqrt(S2/D - mean^2 + eps), nmrs = -mean*rstd
                    sA = lnp.tile([128, 8], dt.float32, tag="sA")
                    nc.sync.dma_start(sA[:], statsR[i][0:128, :])
                    sB = lnp.tile([128, 8], dt.float32, tag="sB")
                    nc.sync.dma_start(sB[:], statsR[i][128:256, :])
                    nc.vector.tensor_add(statsS[:, 8 * i:8 * i + 8], sA[:], sB[:])
                    ssv = statsS[:, 8 * i:8 * i + 8].rearrange("p (m two) -> p m two", two=2)
                    nmean = lnp.tile([128, 4], dt.float32, tag="nmean")
                    varpe = lnp.tile([128, 4], dt.float32, tag="varpe")
                    nc.scalar.mul(nmean[:], ssv[:, :, 0], -1.0 / DIM)
                    nc.vector.tensor_scalar_mul(varpe[:], ssv[:, :, 1], 1.0 / DIM)
                    nc.vector.tensor_mul(rstd4[i][:], nmean[:], nmean[:])
                    nc.vector.tensor_sub(varpe[:], varpe[:], rstd4[i][:])
                    nc.vector.tensor_scalar_add(varpe[:], varpe[:], 1e-5)
                    nc.scalar.activation(rstd4[i][:], varpe[:], F.Sqrt)
                    nc.vector.reciprocal(rstd4[i][:], rstd4[i][:])
                    nc.vector.tensor_mul(nmrs4[i][:], nmean[:], rstd4[i][:])

                # process in m-halves so the first stats exchange (AllGather
                # + local add, ~2x lower latency than an AllReduce) is on
                # the wire while the second half's matmuls run
                p_os = {}

                def ln_finish(m):
                    i, mm = divmod(m, 4)
                    of = lnp.tile([128, 512], dt.bfloat16, tag="of")
                    off = lnp.tile([128, 512], dt.bfloat16, tag="off")
                    nc.scalar.activation(of[:], p_os[m][:], F.Identity,
                                         scale=rstd4[i][:, mm:mm + 1],
                                         bias=nmrs4[i][:, mm:mm + 1])
                    nc.vector.tensor_mul(off[:], of[:], g_bc[:])
                    nc.sync.dma_start(d_out.ap()[m * 128:(m + 1) * 128, :], off[:])

                for half in range(2):
                    ms = range(half * 4, half * 4 + 4)
                    for m in ms:
                        p_os[m] = prj.tile([128, 512], dt.float32, name=f"p_o{m}", tag=f"p_o{m}")
                        for k in range(6):
                            nc.tensor.matmul(p_os[m][:, 0:512],
                                             oT[k][:, m * 128:(m + 1) * 128],
                                             wout_t[k][:, 0:512],
                                             start=(k == 0), stop=False)
                    for m in ms:
                        for k in (6, 7):
                            nc.tensor.matmul(p_os[m][:, 0:512],
                                             oT[k][:, m * 128:(m + 1) * 128],
                                             wout_t[k][:, 0:512],
                                             start=False, stop=(k == 7))
                        sq = lnp.tile([128, 512], dt.bfloat16, tag="sq")
                        nc.vector.tensor_reduce(stats[:, 2 * m:2 * m + 1], p_os[m][:],
                                                mybir.AxisListType.X, OP.add)
                        nc.scalar.activation(sq[:], p_os[m][:], F.Square,
                                             accum_out=stats[:, 2 * m + 1:2 * m + 2])
                    ln_send(half)
                    if half == 0:
                        pass  # math(0) emitted after half 1's k0..5 below
                    else:
                        ln_math(0)
                        for m in range(4):
                            ln_finish(m)
                        ln_math(1)
                        for m in range(4, 8):
                            ln_finish(m)

    nc.finalize()
    return nc


def _prep_inputs(x, rotary_pos_emb, w_qkv, w_out, g):
    cos = np.cos(np.asarray(rotary_pos_emb, np.float32))
    sin = np.sin(np.asarray(rotary_pos_emb, np.float32))
    x = np.asarray(x, np.float32)
    w_qkv = np.asarray(w_qkv, np.float32)
    w_out = np.asarray(w_out, np.float32)
    g = np.asarray(g, np.float32)

    # gathered inner-dim order after the four per-t4 AllGathers:
    # k-tile 2*t4   = even core's heads (2*t4, 2*t4+1)
    # k-tile 2*t4+1 = odd  core's heads (8+2*t4, 8+2*t4+1)
    perm = np.r_[0:128, 512:640, 128:256, 640:768,
                 256:384, 768:896, 384:512, 896:1024]
    wout_p = np.ascontiguousarray(w_out[perm])

    def rope_pair(c_arr, s_arr, cols):
        # transposed layout, one 32-row head slot (the kernel DMAs it into
        # both); sin sign-folded for rotate_half
        cc = c_arr.T.astype(bf16)
        ss = np.zeros((32, cols), bf16)
        ss[0:HALF] = (-s_arr.T[0:HALF]).astype(bf16)
        ss[HALF:32] = s_arr.T[HALF:32].astype(bf16)
        return np.ascontiguousarray(cc), ss

    tri = np.tril(np.ones((128, 128), np.float32)).T  # [key, q]: 1 if key<=q

    in_maps = []
    for c in range(NCORES):
        b, s, h = c // 4, (c // 2) % 2, c % 2
        pord = BLOCKS[s] + BLOCKS[1 - s]
        kperm = np.concatenate([np.arange(bl * 128, (bl + 1) * 128) for bl in pord])
        # tile-contiguous: [(c2, k), 128, 1024] so each load is one block
        xT_in = np.ascontiguousarray(
            x[b][kperm].T.astype(bf16).reshape(8, 128, 2, 1024)
            .transpose(2, 0, 1, 3).reshape(2048, 1024))
        cosK_in, sinK_in = rope_pair(cos[kperm], sin[kperm], N)
        # v rotary tables in key-chunk order: [p, (ci, d)]; sign-folded sin
        cr = cos[kperm].reshape(16, 128, 32).transpose(1, 0, 2)
        sr = sin[kperm].reshape(16, 128, 32).transpose(1, 0, 2).copy()
        sr[:, :, 0:HALF] *= -1.0
        cosVs = np.ascontiguousarray(cr.reshape(128, 512)).astype(bf16)
        sinVs = np.ascontiguousarray(sr.reshape(128, 512)).astype(bf16)
        hs = slice(h * 512, (h + 1) * 512)
        wq_in = (w_qkv[:, 0:1024][:, hs] * SCALE).astype(bf16)
        wkv_in = np.concatenate(
            [w_qkv[:, 1024:2048][:, hs], w_qkv[:, 2048:3072][:, hs]], axis=1
        ).astype(bf16)
        # tile-contiguous: [k, 128, 1024]
        wkv_in = np.ascontiguousarray(wkv_in)
        # diagonal masks: own blocks get tri; partner diagonal chunk is
        # fully masked on even-q cores, fully visible on odd-q cores
        maskD_in = np.zeros((128, 256), np.float32)
        maskD_in[:, 0:128] = tri
        maskD_in[:, 128:256] = 0.0 if s == 0 else 1.0
        in_maps.append({
            "xT": xT_in,
            "wkv": wkv_in,
            "wq": wq_in,
            "wout": wout_p[:, h * 512:(h + 1) * 512].astype(bf16),
            "cosK": cosK_in, "sinK": sinK_in,
            "cosVs": cosVs, "sinVs": sinVs,
            "maskD": maskD_in.astype(bf16),
            "gw": g[h * 512:(h + 1) * 512].reshape(1, 512).astype(bf16),
        })
    return in_maps


def _maybe_install_trace_hook():
    import sys, types
    try:
        import antenv.axon_hooks  # noqa: F401
        return
    except ImportError:
        pass
    try:
        import trn_agent_boot.trn_boot as tb
        hook = tb._ntff_profile_via_ctypes('/opt/axon/libaxon_pjrt.so')
        mod = types.ModuleType('antenv.axon_hooks')
        mod.get_axon_ntff_profile_hook = lambda: hook
        mod.set_axon_ntff_profile_hook = lambda h: None
        sys.modules['antenv.axon_hooks'] = mod
    except Exception:
        pass


def kernel(x, mask, rotary_pos_emb, w_qkv, w_out, g):
    global LAST_EXEC_NS, LAST_RESULTS
    if "nc" not in _CACHE:
        _CACHE["nc"] = _build_program()
    nc = _CACHE["nc"]
    in_maps = _prep_inputs(x, rotary_pos_emb, w_qkv, w_out, g)
    kwargs = {}
    if TRACE:
        _maybe_install_trace_hook()
        kwargs = dict(trace=True, trace_cores=list(range(NCORES)))
    res = run_bass_kernel_spmd(nc, in_maps, core_ids=list(range(NCORES)), **kwargs)
    LAST_EXEC_NS = res.exec_time_ns
    LAST_RESULTS = res
    out = np.empty((B, N, DIM), np.float32)
    for b in range(B):
        for s in range(2):
            qperm = np.concatenate(
                [np.arange(bl * 128, (bl + 1) * 128) for bl in BLOCKS[s]])
            for h in range(2):
                core = b * 4 + s * 2 + h
                r = np.asarray(res.results[core]["out"], np.float32)
                out[b, qperm, h * 512:(h + 1) * 512] = r
    return out


# revision 70
# speedup vs baseline: 1.0217x; 1.0217x over previous
"""Trainium2 Bass kernel for a dense causal-attention transformer block.

Full computation (matches the fp32 jax reference):
    qkv = x @ w_qkv ; split q,k,v ; heads 16x64 ; q *= 64**-0.5
    rotary (first 32 dims) applied to q, k AND v
    causal softmax attention ; merge heads ; @ w_out ; LayerNorm(g)

Sharding over 8 NeuronCores: core = b*4 + s*2 + h
    b: batch (2)   s: sequence half (even/odd 128-row blocks)   h: head half
Each core ships x with columns in ITS key order P = [own-q-desc ++
partner-q-desc] so (a) the Q projection is just the first 1024 columns of
xT (no duplicate load) and (b) every key block's visible q-set is a
contiguous prefix, uniform across cores; the residual s-asymmetry of the
odd/even split lives in a tiny [128,256] diagonal-mask input.
Cross-core exchange: four 2-core AllGathers (0.25 MB bf16 each) of the
per-head-pair attention outputs, issued as each head pair finishes; the
gathered tiles are copied to SBUF only after the attention loop so a
skew-delayed collective can never block the DMA queue mid-attention.
The output projection is COLUMN-split across the pair (each core holds its
512 columns of w_out), and LayerNorm statistics are reconstructed from an
8 KB AllReduce of per-row partial sums (sum, sum-of-squares).

All matmuls in bf16 with fp32 PSUM accumulation; softmax in fp32 without
max-subtraction (scores are O(5), exp is safe); mask=ones per the input spec.
"""

import numpy as np
import ml_dtypes

import concourse.bass as bass
import concourse.mybir as mybir
import concourse.tile as tile
from concourse import bacc
from concourse.bass_utils import run_bass_kernel_spmd

dt = mybir.dt
F = mybir.ActivationFunctionType
OP = mybir.AluOpType
bf16 = ml_dtypes.bfloat16

B, N, DIM = 2, 2048, 1024
HEADS, DH, ROT = 16, 64, 32
HALF = ROT // 2  # 16
SCALE = DH ** -0.5
NCORES = 8
NB = N // 128            # 16 global q/k blocks
BLOCKS = {0: [14, 12, 10, 8, 6, 4, 2, 0], 1: [15, 13, 11, 9, 7, 5, 3, 1]}
PAIRS = [[0, 1], [2, 3], [4, 5], [6, 7]]
# attention iteration order per head pair: all OWN key blocks first (their
# K/V projections only need the first half of xT, so attention starts while
# the second half still streams in), then the partner blocks
ITER = [(j, 0) for j in range(7, -1, -1)] + [(j, 1) for j in range(7, -1, -1)]
HI_STOP = ITER.index((4, 1))   # last iteration touching psum cols 512:1024

TRACE = False
LAST_EXEC_NS = None
LAST_RESULTS = None

_CACHE = {}


def _build_program():
    nc = bacc.Bacc(trn_type="TRN2", target_bir_lowering=False, debug=False,
                   num_devices=NCORES)

    # activation/weight tensors are stored TILE-CONTIGUOUS on the host
    # (each tile one contiguous block) so load DMAs use wide per-partition
    # rows (2-4 KB descriptors instead of 1 KB)
    d_xT = nc.dram_tensor("xT", [16 * 128, 1024], dt.bfloat16, kind="ExternalInput")
    d_wkv = nc.dram_tensor("wkv", [8 * 128, 1024], dt.bfloat16, kind="ExternalInput")
    d_wq = nc.dram_tensor("wq", [8 * 128, 512], dt.bfloat16, kind="ExternalInput")
    d_wout = nc.dram_tensor("wout", [8 * 128, 512], dt.bfloat16, kind="ExternalInput")
    # rope tables: both 32-row head slots hold identical data; ship once
    d_cosK = nc.dram_tensor("cosK", [32, N], dt.bfloat16, kind="ExternalInput")
    d_sinK = nc.dram_tensor("sinK", [32, N], dt.bfloat16, kind="ExternalInput")
    d_cosVs = nc.dram_tensor("cosVs", [128, 512], dt.bfloat16, kind="ExternalInput")
    d_sinVs = nc.dram_tensor("sinVs", [128, 512], dt.bfloat16, kind="ExternalInput")
    d_maskD = nc.dram_tensor("maskD", [128, 256], dt.bfloat16, kind="ExternalInput")
    d_g = nc.dram_tensor("gw", [1, 512], dt.bfloat16, kind="ExternalInput")
    d_out = nc.dram_tensor("out", [1024, 512], dt.bfloat16, kind="ExternalOutput")

    with tile.TileContext(nc) as tc:
        with (
            tc.tile_pool(name="cst", bufs=1) as cst,
            tc.tile_pool(name="dram", bufs=1, space="DRAM") as dpool,
        ):
            # ---- persistent SBUF tiles
            kT = cst.tile([128, 4 * N], dt.bfloat16, tag="kT")       # [2head-dims, key slots]
            vA = cst.tile([128, 16 * 520], dt.bfloat16, tag="vA")    # rows; per chunk 8x(64 dims + 1 one)
            qT = cst.tile([128, 4 * 1024], dt.bfloat16, tag="qT")
            outT = cst.tile([128, 4 * 1024], dt.bfloat16, tag="outT")
            cosK = cst.tile([128, N], dt.bfloat16, tag="cosK")
            sinK = cst.tile([128, N], dt.bfloat16, tag="sinK")
            maskD = cst.tile([128, 256], dt.bfloat16, tag="maskD")
            g_bc = cst.tile([128, 512], dt.bfloat16, tag="g_bc")
            g_row = cst.tile([1, 512], dt.bfloat16, tag="g_row")
            eps_t = cst.tile([128, 1], dt.float32, tag="eps_t")
            wout_t = [cst.tile([128, 512], dt.bfloat16, name=f"wout{k}", tag=f"wout{k}") for k in range(8)]
            oT = [cst.tile([128, 1024], dt.bfloat16, name=f"oT{k}", tag=f"oT{k}") for k in range(8)]

            agin4 = [dpool.tile([128, 1024], dt.bfloat16, name=f"agin{t}") for t in range(4)]
            agout4 = [dpool.tile([256, 1024], dt.bfloat16, name=f"agout{t}") for t in range(4)]
            statsD = [dpool.tile([128, 8], dt.float32, name=f"statsD{i}") for i in range(2)]
            statsR = [dpool.tile([256, 8], dt.float32, name=f"statsR{i}") for i in range(2)]

            # alternate input loads across the two HW DMA queues (SP + ACT):
            # the Sync sequencer takes ~600ns per DMA issue, so a single
            # queue serializes the load phase
            def load(i, dst, src):
                (nc.sync if i % 2 == 0 else nc.scalar).dma_start(dst, src)

            # ================= projections =================
            with (
                tc.tile_pool(name="xw", bufs=1) as xw,
                tc.tile_pool(name="rotp", bufs=1) as rotp,
                tc.tile_pool(name="simp", bufs=2, space="PSUM") as simp,
                tc.tile_pool(name="outp", bufs=1, space="PSUM") as outp,
            ):
                xT_t = [xw.tile([128, N], dt.bfloat16, name=f"xT{k}", tag=f"xT{k}") for k in range(8)]
                wkv_t = [xw.tile([128, 1024], dt.bfloat16, name=f"wkv{k}", tag=f"wkv{k}") for k in range(8)]
                wq_t = [xw.tile([128, 512], dt.bfloat16, name=f"wq{k}", tag=f"wq{k}") for k in range(8)]

                # --- input DMAs in consumption order
                def tload(i, dst, dten, t):
                    load(i, dst, dten.ap()[t * 128:(t + 1) * 128, :])

                for k in range(8):
                    tload(k, wkv_t[k][:, 0:1024], d_wkv, k)
                for k in range(8):
                    tload(k, xT_t[k][:, 0:1024], d_xT, k)
                for k in range(8):
                    tload(k, wq_t[k][:], d_wq, k)
                for lo in (0, 64):
                    load(0, cosK[lo:lo + 32, :], d_cosK.ap()[:])
                    load(1, sinK[lo:lo + 32, :], d_sinK.ap()[:])
                vrot = tc.alloc_tile_pool(name="vrot", bufs=1)
                cosV = vrot.tile([128, 4096], dt.bfloat16, tag="cosV")
                sinV = vrot.tile([128, 4096], dt.bfloat16, tag="sinV")
                cosVs = vrot.tile([128, 512], dt.bfloat16, tag="cosVs")
                sinVs = vrot.tile([128, 512], dt.bfloat16, tag="sinVs")
                load(0, cosVs[:], d_cosVs.ap()[:])
                load(1, sinVs[:], d_sinVs.ap()[:])
                cV4 = cosV[:, 0:4096].rearrange("p (c h e) -> p c h e", c=16, h=8)
                sV4 = sinV[:, 0:4096].rearrange("p (c h e) -> p c h e", c=16, h=8)
                # broadcast the per-position rotary tables across the 8 head
                # slots on the ACT engine (a strided DMA would shatter into
                # 64-byte descriptors)
                for h8 in range(8):
                    nc.scalar.copy(cV4[:, :, h8, :],
                                   cosVs[:, :].rearrange("p (c e) -> p c e", c=16))
                    nc.scalar.copy(sV4[:, :, h8, :],
                                   sinVs[:, :].rearrange("p (c e) -> p c e", c=16))
                load(0, maskD[:], d_maskD.ap()[:])
                load(1, g_row[:], d_g.ap()[:])
                for k in range(8):
                    tload(k, xT_t[k][:, 1024:2048], d_xT, 8 + k)
                for k in range(8):
                    tload(k, wout_t[k][:], d_wout, k)
                nc.gpsimd.partition_broadcast(g_bc[:], g_row[:])
                nc.vector.memset(eps_t[:], 1e-5)

                # ones column of the [v | 1] PV weights (col 64 of each 65-slot)
                nc.vector.memset(
                    vA[:, 0:16 * 520].rearrange("p (c h e) -> p c h e", c=16, h=8)[:, :, :, 64:65],
                    1.0,
                )

                # rotary for a column window of a head-pair tile, in place:
                # swap the 16-row halves via SBUF->SBUF DMA, then
                # t' = t*cos + swapped*sin_signed on the {0:32},{64:96} rows.
                # The DVE multiplies are split into 512-column chunks so the
                # burst can be spread across attention iterations instead of
                # blocking the mask chain.
                def rot_swap(t, t4, width, c0, cw, tag):
                    c = slice(t4 * width + c0, t4 * width + c0 + cw)
                    tmp = rotp.tile([128, cw], dt.bfloat16, tag=tag)
                    for lo in (0, 64):
                        nc.sync.dma_start(tmp[lo:lo + 16, :], t[lo + 16:lo + 32, c])
                        nc.sync.dma_start(tmp[lo + 16:lo + 32, :], t[lo:lo + 16, c])
                    return tmp

                def rot_mul(t, t4, width, tmp, c0, d0, dw):
                    cc = slice(t4 * width + c0 + d0, t4 * width + c0 + d0 + dw)
                    cl = slice(c0 + d0, c0 + d0 + dw)  # cosK/sinK columns
                    tl = slice(d0, d0 + dw)
                    for lo in (0, 64):
                        sl = slice(lo, lo + 32)
                        nc.vector.tensor_mul(tmp[sl, tl], tmp[sl, tl], sinK[sl, cl])
                        nc.vector.tensor_mul(t[sl, cc], t[sl, cc], cosK[sl, cl])
                        nc.vector.tensor_add(t[sl, cc], t[sl, cc], tmp[sl, tl])

                def make_rot_own(m4):
                    # own-half kT (cols 0:1024 of the t4 tile) + all of qT:
                    # a swap thunk and four 512-col multiply thunks
                    st = {}

                    def do_swap():
                        st['ko'] = rot_swap(kT, m4, N, 0, 1024, "rko")
                        st['q'] = rot_swap(qT, m4, 1024, 0, 1024, "rq")
                    # ordered by first use in the next phase: all of qT at
                    # it=0, kT cols 512:1024 (slots 7..4) before 0:512
                    muls = [lambda: rot_mul(qT, m4, 1024, st['q'], 0, 0, 512),
                            lambda: rot_mul(qT, m4, 1024, st['q'], 0, 512, 512),
                            lambda: rot_mul(kT, m4, N, st['ko'], 0, 512, 512),
                            lambda: rot_mul(kT, m4, N, st['ko'], 0, 0, 512)]
                    return do_swap, muls

                def make_rot_part(m4):
                    st = {}

                    def do_swap():
                        st['kp'] = rot_swap(kT, m4, N, 1024, 1024, "rkp")
                    muls = [lambda: rot_mul(kT, m4, N, st['kp'], 1024, 0, 512),
                            lambda: rot_mul(kT, m4, N, st['kp'], 1024, 512, 512)]
                    return do_swap, muls

                # one kT or qT projection psum-group; shares the simp PSUM
                # ring with attention so groups for head pair m4 can be
                # interleaved into head pair m4-1's attention, keeping the
                # PE dense
                def emit_kq_group(m4, gi, on_act=False):
                    cp = nc.scalar.copy if on_act else nc.vector.tensor_copy
                    ps = simp.tile([128, 1024], dt.float32, tag="s_ps", name=f"pjg{m4}_{gi}")
                    if gi < 4:
                        for k in range(8):
                            nc.tensor.matmul(ps[:, 0:512], wkv_t[k][:, m4 * 128:(m4 + 1) * 128],
                                             xT_t[k][:, gi * 512:(gi + 1) * 512],
                                             start=(k == 0), stop=(k == 7))
                        cp(kT[:, m4 * N + gi * 512: m4 * N + (gi + 1) * 512], ps[:, 0:512])
                    else:
                        nn = gi - 4
                        for k in range(8):
                            nc.tensor.matmul(ps[:, 0:512], wq_t[k][:, m4 * 128:(m4 + 1) * 128],
                                             xT_t[k][:, nn * 512:(nn + 1) * 512],
                                             start=(k == 0), stop=(k == 7))
                        cp(qT[:, m4 * 1024 + nn * 512: m4 * 1024 + (nn + 1) * 512], ps[:, 0:512])


                # v chunk r: natural rows x (8 heads x 64), strided into
                # 65-slots; copies on ACT (DVE is loaded with rotary)
                tmpV = vrot.tile([128, 4 * 256], dt.bfloat16, tag="tmpV")
                v4 = vA[:, 0:16 * 520].rearrange("p (c h e) -> p c h e", c=16, h=8)
                t4v = tmpV[:, 0:4 * 256].rearrange("p (c h e) -> p c h e", c=4, h=8)

                def emit_v_chunk(r, rot):
                    ps = simp.tile([128, 1024], dt.float32, tag="s_ps", name=f"vps{r}")
                    for k in range(8):
                        nc.tensor.matmul(ps[:, 0:512], xT_t[k][:, r * 128:(r + 1) * 128],
                                         wkv_t[k][:, 512:1024],
                                         start=(k == 0), stop=(k == 7))
                    nc.scalar.copy(
                        vA[:, r * 520: r * 520 + 520].rearrange("p (h e) -> p h e", h=8)[:, :, 0:64],
                        ps[:, 0:512].rearrange("p (h e) -> p h e", h=8),
                    )
                    if rot is not None:
                        # tmp[a] = v[b]*sinS[a]; tmp[b] = v[a]*sinS[b]
                        sl = rot
                        nw = sl.stop - sl.start
                        tv = t4v[:, 0:nw]
                        nc.vector.tensor_mul(tv[:, :, :, 0:16], v4[:, sl, :, 16:32], sV4[:, sl, :, 0:16])
                        nc.vector.tensor_mul(tv[:, :, :, 16:32], v4[:, sl, :, 0:16], sV4[:, sl, :, 16:32])
                        nc.vector.tensor_mul(v4[:, sl, :, 0:32], v4[:, sl, :, 0:32], cV4[:, sl, :, 0:32])
                        nc.vector.tensor_add(v4[:, sl, :, 0:32], v4[:, sl, :, 0:32], tv[:, :, :, 0:32])

                # projection order puts everything that only needs the first
                # half of xT (key slots 0..7 = the core's own q columns)
                # first: own-K, Q, own rotary, own-V (descending, the order
                # attention consumes them).  Partner-half projections (gi
                # 2,3), their rotary, and v chunks 15..8 stream into the
                # first attention phase's slots.
                for gi in (0, 1, 4, 5):
                    emit_kq_group(0, gi, on_act=True)
                swap0, muls0 = make_rot_own(0)
                swap0()
                for th in muls0:
                    th()
                for r in (7, 6, 5, 4, 3, 2):
                    emit_v_chunk(r, slice(r, r + 1))

                # ================= attention =================
                # key slot for (j, par): own prefix-j at slot j, partner at 8+j
                att = tc.alloc_tile_pool(name="att", bufs=7)
                nrm = tc.alloc_tile_pool(name="nrm", bufs=2)
                nrm1 = tc.alloc_tile_pool(name="nrm1", bufs=2)

                # normalize: rows 0..63 = sum(exp*v), row 64 = sum(exp).
                # Copy PSUM out fast (so the next head pair's PV — or the
                # out-projection, for the last pair — can reuse the
                # accumulator banks), then 1/s on DVE with the sums row
                # spread across 128 partitions via a DRAM bounce
                # (single-partition DVE ops cost ~3.3us; spread ones are
                # ~0.1us); partition-broadcast on GpSimd.  The final
                # per-head 1/s multiplies (and the AllGather behind them)
                # are returned as deferred thunks: running them inside the
                # NEXT phase keeps the DVE queue free for its mask chain.
                def norm(t4, o_2, pool=None, pool1=None, sfx="", eng=None):
                    pool = pool or nrm
                    pool1 = pool1 or nrm1
                    eng = eng or nc.sync
                    osb = []
                    for hsi in range(2):
                        ot = pool.tile([65, 1024], dt.bfloat16, name=f"osb{hsi}{sfx}",
                                       tag=f"osb{hsi}{sfx}")
                        nc.vector.tensor_copy(ot[:], o_2[hsi][:])
                        osb.append(ot)
                    sh = pool1.tile([128, 16], dt.bfloat16, tag="sh" + sfx)
                    shf = pool1.tile([128, 16], dt.float32, tag="shf" + sfx)
                    shr = pool1.tile([128, 16], dt.bfloat16, tag="shr" + sfx)
                    for hsi in range(2):
                        dsum = dpool.tile([1, 1024], dt.bfloat16, name=f"dsum{t4}_{hsi}")
                        eng.dma_start(dsum[:], osb[hsi][64:65, :])
                        eng.dma_start(
                            sh[:, hsi * 8:(hsi + 1) * 8],
                            dsum[0:1, :].rearrange("o (p c) -> (o p) c", p=128))
                    nc.vector.tensor_copy(shf[:], sh[:])
                    nc.vector.reciprocal(shf[:], shf[:])
                    nc.vector.tensor_copy(shr[:], shf[:])
                    thunks = []
                    for hsi in range(2):
                        def mul_thunk(hsi=hsi):
                            hb = hsi * 64
                            drs = dpool.tile([1, 1024], dt.bfloat16, name=f"drs{t4}_{hsi}")
                            rs2 = pool1.tile([1, 1024], dt.bfloat16, tag="rs2" + sfx)
                            bc = pool1.tile([64, 1024], dt.bfloat16, tag="bc" + sfx)
                            eng.dma_start(
                                drs[0:1, :].rearrange("o (p c) -> (o p) c", p=128),
                                shr[:, hsi * 8:(hsi + 1) * 8])
                            eng.dma_start(rs2[:], drs[:])
                            nc.gpsimd.partition_broadcast(bc[:], rs2[:])
                            nc.vector.tensor_mul(outT[hb:hb + 64, t4 * 1024:(t4 + 1) * 1024],
                                                 osb[hsi][0:64, :], bc[:])
                        thunks.append(mul_thunk)
                    thunks.append(lambda: ship(t4))
                    return thunks

                def ship(t4, eng=None):
                    # ship this head-pair's output to the partner; the
                    # transfer hides under the remaining attention
                    (eng or nc.sync).dma_start(agin4[t4][:], outT[:, t4 * 1024:(t4 + 1) * 1024])
                    nc.gpsimd.collective_compute(
                        "AllGather", OP.bypass, replica_groups=PAIRS,
                        ins=[agin4[t4].opt()], outs=[agout4[t4].opt()],
                    )

                deferred = []
                for t4 in range(4):
                    # both heads of the pair interleaved; PV of iteration it
                    # is emitted after QK/exp of it+2 (two-block software
                    # pipeline) so the PE never waits on the exp chain
                    o_2 = [outp.tile([65, 1024], dt.float32, name=f"o2_{t4}_{i}", tag=f"o2_{i}")
                           for i in range(2)]

                    def emit_pv(it, hsi, w, eT, t4=t4, o_2=o_2):
                        slot = ITER[it][0] + 8 * ITER[it][1]
                        lh = t4 * 2 + hsi
                        for c0 in range(0, w, 512):
                            cw = min(512, w - c0)
                            nc.tensor.matmul(
                                o_2[hsi][:, c0:c0 + cw],
                                vA[:, slot * 520 + lh * 65: slot * 520 + lh * 65 + 65],
                                eT[:, c0:c0 + cw],
                                start=(it == 0),
                                stop=(it == (15 if c0 == 0 else HI_STOP)))

                    # per-iteration side work, scheduled so that by the time
                    # the NEXT phase starts its own-key half, that half's
                    # projections + rotary are done, and this phase's
                    # partner-key half (it>=8) has ITS dependencies done:
                    #   t4=0 only: gi 2,3 + partner rotary of pair 0, and v
                    #     chunks 15..8, early
                    #   all t4<3: next pair's 6 projection groups, own+q
                    #     rotary mid-phase, partner rotary at the end
                    #   previous phase's deferred 1/s multiplies + AllGather
                    #     at 1/3/5
                    slots = {}
                    spill = []

                    def sched(it_s, th):
                        if it_s is None:
                            spill.append(th)
                        else:
                            slots.setdefault(it_s, []).append(th)

                    for i, th in enumerate(deferred):
                        sched(1 + 2 * i, th)
                    if t4 == 0:
                        # remaining own-v feeding the A-half PVs just in
                        # time; this pair's partner projections (xT second
                        # half lands mid-phase) + partner rotary before
                        # it=8; partner v; next pair's work late
                        sched(0, lambda: emit_v_chunk(1, slice(1, 2)))
                        sched(1, lambda: emit_v_chunk(0, slice(0, 1)))
                        sched(4, lambda: emit_kq_group(0, 2))
                        swp, mulp = make_rot_part(0)
                        sched(5, lambda: emit_kq_group(0, 3))
                        sched(5, swp)
                        sched(6, mulp[0])
                        sched(7, mulp[1])
                        for i, r in enumerate(range(15, 7, -1)):
                            sched(6 + i, lambda r=r: emit_v_chunk(r, slice(r, r + 1)))
                        kq_sl = (8, 9, 10, 11, 12, 13)
                        own_sl = (14, 14, 15, 15, None)
                        par_sl = (None, None, None)
                    else:
                        kq_sl = (2, 3, 4, 5, 6, 7)
                        own_sl = (6, 7, 8, 9, 10)
                        par_sl = (13, 14, 15)
                    if t4 < 3:
                        m4 = t4 + 1
                        for it_s, gi in zip(kq_sl, (0, 1, 4, 5, 2, 3)):
                            sched(it_s, lambda gi=gi, m4=m4: emit_kq_group(m4, gi))
                        swo, mulo = make_rot_own(m4)
                        sched(own_sl[0], swo)
                        for it_s, th in zip(own_sl[1:], mulo):
                            sched(it_s, th)
                        swp2, mulp2 = make_rot_part(m4)
                        sched(par_sl[0], swp2)
                        sched(par_sl[1], mulp2[0])
                        sched(par_sl[2], mulp2[1])

                    pend = []
                    for it, (j, par) in enumerate(ITER):
                        slot = j + 8 * par
                        w = (j + 1) * 128
                        sps2 = []
                        for hsi in range(2):
                            hb = hsi * 64
                            s_ps = simp.tile([128, 1024], dt.float32, tag="s_ps")
                            for c0 in range(0, w, 512):
                                cw = min(512, w - c0)
                                nc.tensor.matmul(
                                    s_ps[:, c0:c0 + cw],
                                    kT[hb:hb + 64, t4 * N + slot * 128: t4 * N + (slot + 1) * 128],
                                    qT[hb:hb + 64, t4 * 1024 + c0: t4 * 1024 + c0 + cw],
                                    start=True, stop=True)
                            sps2.append(s_ps)
                        for hsi in range(2):
                            eT = att.tile([128, 1024], dt.bfloat16, tag="eT")
                            nc.scalar.activation(eT[:, 0:w], sps2[hsi][:, 0:w], F.Exp)
                            nc.vector.tensor_mul(eT[:, w - 128:w], eT[:, w - 128:w],
                                                 maskD[:, par * 128:(par + 1) * 128])
                            pend.append((it, hsi, w, eT))
                        while len(pend) > 4:
                            emit_pv(*pend.pop(0))
                        for th in slots.get(it, []):
                            th()
                    for p in pend:
                        emit_pv(*p)

                    if t4 < 3:
                        deferred = norm(t4, o_2)
                    else:
                        # last pair: normalize immediately (its latency chain
                        # gates the final AllGather), on PERSISTENT tiles so
                        # the attention pools' release boundaries (and the
                        # out-projection's PSUM alloc behind them) never wait
                        # for it.  No DRAM bounce here: the single-partition
                        # DVE reciprocal (~3.3us) beats the bounce's ~10us
                        # of DMA latency on this exposed chain.  ship(3) is
                        # emitted AFTER the pool transition below.
                        osb3 = []
                        for hsi in range(2):
                            ot3 = cst.tile([65, 1024], dt.bfloat16, tag=f"osb{hsi}3")
                            nc.vector.tensor_copy(ot3[:], o_2[hsi][:])
                            osb3.append(ot3)
                        for hsi in range(2):
                            rec3 = cst.tile([1, 1024], dt.bfloat16, tag=f"rec{hsi}3")
                            bc3 = cst.tile([64, 1024], dt.bfloat16, tag=f"bc{hsi}3")
                            with nc.allow_low_precision(
                                    reason="1/s in bf16, same as the bounce path"):
                                nc.vector.reciprocal(rec3[:], osb3[hsi][64:65, :])
                            nc.gpsimd.partition_broadcast(bc3[:], rec3[:])
                            nc.vector.tensor_mul(
                                outT[hsi * 64:hsi * 64 + 64, 3072:4096],
                                osb3[hsi][0:64, :], bc3[:])
                    for th in spill:
                        th()
                # pull the first three pairs' gathered tiles to SBUF only
                # now: a skew-delayed AllGather can never block the DMA
                # queue behind attention-phase DMAs
                for p4 in range(3):
                    nc.sync.dma_start(oT[2 * p4][:], agout4[p4][0:128, :])
                    nc.sync.dma_start(oT[2 * p4 + 1][:], agout4[p4][128:256, :])
                nrm1.release()
                nrm.release()
                att.release()
                vrot.release()

            # ======== out projection (column-split) + layernorm ========
            # k-tiles 6,7 come from the last AllGather; accumulate them last
            # so the first k0..5 matmuls cover its latency.  LN statistics
            # (sum, sum of squares over this core's 512 columns) are summed
            # with the partner's via a tiny AllReduce.
            with (
                tc.tile_pool(name="prj", bufs=1, space="PSUM") as prj,
                tc.tile_pool(name="lnp", bufs=2) as lnp,
                tc.tile_pool(name="lns", bufs=1) as lns,
            ):
                ship(3, eng=nc.scalar)
                stats = lns.tile([128, 16], dt.float32, tag="stats")
                statsS = lns.tile([128, 16], dt.float32, tag="statsS")
                rstd4 = [lns.tile([128, 4], dt.float32, name=f"rstd4_{i}", tag=f"rstd4_{i}")
                         for i in range(2)]
                nmrs4 = [lns.tile([128, 4], dt.float32, name=f"nmrs4_{i}", tag=f"nmrs4_{i}")
                         for i in range(2)]
                nc.sync.dma_start(oT[6][:], agout4[3][0:128, :])
                nc.sync.dma_start(oT[7][:], agout4[3][128:256, :])

                def ln_send(i):
                    nc.sync.dma_start(statsD[i][:], stats[:, 8 * i:8 * i + 8])
                    nc.gpsimd.collective_compute(
                        "AllGather", OP.bypass, replica_groups=PAIRS,
                        ins=[statsD[i].opt()], outs=[statsR[i].opt()],
                    )

                def ln_math(i):
                    # batched LN math for row-blocks 4i..4i+3:
                    # rstd = 1/sqrt(S2/D - mean^2 + eps), nmrs = -mean*rstd
                    sA = lnp.tile([128, 8], dt.float32, tag="sA")
                    nc.sync.dma_start(sA[:], statsR[i][0:128, :])
                    sB = lnp.tile([128, 8], dt.float32, tag="sB")
                    nc.sync.dma_start(sB[:], statsR[i][128:256, :])
                    nc.vector.tensor_add(statsS[:, 8 * i:8 * i + 8], sA[:], sB[:])
                    ssv = statsS[:, 8 * i:8 * i + 8].rearrange("p (m two) -> p m two", two=2)
                    nmean = lnp.tile([128, 4], dt.float32, tag="nmean")
                    varpe = lnp.tile([128, 4], dt.float32, tag="varpe")
                    nc.scalar.mul(nmean[:], ssv[:, :, 0], -1.0 / DIM)
                    nc.vector.tensor_scalar_mul(varpe[:], ssv[:, :, 1], 1.0 / DIM)
                    nc.vector.tensor_mul(rstd4[i][:], nmean[:], nmean[:])
                    nc.vector.tensor_sub(varpe[:], varpe[:], rstd4[i][:])
                    nc.vector.tensor_scalar_add(varpe[:], varpe[:], 1e-5)
                    nc.scalar.activation(rstd4[i][:], varpe[:], F.Sqrt)
                    nc.vector.reciprocal(rstd4[i][:], rstd4[i][:])
                    nc.vector.tensor_mul(nmrs4[i][:], nmean[:], rstd4[i][:])

                # process in m-halves so the first stats exchange (AllGather
                # + local add, ~2x lower latency than an AllReduce) is on
                # the wire while the second half's matmuls run
                p_os = {}

                def ln_finish(m):
                    i, mm = divmod(m, 4)
                    of = lnp.tile([128, 512], dt.bfloat16, tag="of")
                    off = lnp.tile([128, 512], dt.bfloat16, tag="off")
                    nc.scalar.activation(of[:], p_os[m][:], F.Identity,
                                         scale=rstd4[i][:, mm:mm + 1],
                                         bias=nmrs4[i][:, mm:mm + 1])
                    nc.vector.tensor_mul(off[:], of[:], g_bc[:])
                    nc.sync.dma_start(d_out.ap()[m * 128:(m + 1) * 128, :], off[:])

                for half in range(2):
                    ms = range(half * 4, half * 4 + 4)
                    for m in ms:
                        p_os[m] = prj.tile([128, 512], dt.float32, name=f"p_o{m}", tag=f"p_o{m}")
                        for k in range(6):
                            nc.tensor.matmul(p_os[m][:, 0:512],
                                             oT[k][:, m * 128:(m + 1) * 128],
                                             wout_t[k][:, 0:512],
                                             start=(k == 0), stop=False)
                    for m in ms:
                        for k in (6, 7):
                            nc.tensor.matmul(p_os[m][:, 0:512],
                                             oT[k][:, m * 128:(m + 1) * 128],
                                             wout_t[k][:, 0:512],
                                             start=False, stop=(k == 7))
                        sq = lnp.tile([128, 512], dt.bfloat16, tag="sq")
                        nc.vector.tensor_reduce(stats[:, 2 * m:2 * m + 1], p_os[m][:],
                                                mybir.AxisListType.X, OP.add)
                        nc.scalar.activation(sq[:], p_os[m][:], F.Square,
                                             accum_out=stats[:, 2 * m + 1:2 * m + 2])
                    ln_send(half)
                    if half == 0:
                        pass  # math(0) emitted after half 1's k0..5 below
                    else:
                        ln_math(0)
                        for m in range(4):
                            ln_finish(m)
                        ln_math(1)
                        for m in range(4, 8):
                            ln_finish(m)

    nc.finalize()
    return nc


def _prep_inputs(x, rotary_pos_emb, w_qkv, w_out, g):
    cos = np.cos(np.asarray(rotary_pos_emb, np.float32))
    sin = np.sin(np.asarray(rotary_pos_emb, np.float32))
    x = np.asarray(x, np.float32)
    w_qkv = np.asarray(w_qkv, np.float32)
    w_out = np.asarray(w_out, np.float32)
    g = np.asarray(g, np.float32)

    # gathered inner-dim order after the four per-t4 AllGathers:
    # k-tile 2*t4   = even core's heads (2*t4, 2*t4+1)
    # k-tile 2*t4+1 = odd  core's heads (8+2*t4, 8+2*t4+1)
    perm = np.r_[0:128, 512:640, 128:256, 640:768,
                 256:384, 768:896, 384:512, 896:1024]
    wout_p = np.ascontiguousarray(w_out[perm])

    def rope_pair(c_arr, s_arr, cols):
        # transposed layout, one 32-row head slot (the kernel DMAs it into
        # both); sin sign-folded for rotate_half
        cc = c_arr.T.astype(bf16)
        ss = np.zeros((32, cols), bf16)
        ss[0:HALF] = (-s_arr.T[0:HALF]).astype(bf16)
        ss[HALF:32] = s_arr.T[HALF:32].astype(bf16)
        return np.ascontiguousarray(cc), ss

    tri = np.tril(np.ones((128, 128), np.float32)).T  # [key, q]: 1 if key<=q

    in_maps = []
    for c in range(NCORES):
        b, s, h = c // 4, (c // 2) % 2, c % 2
        pord = BLOCKS[s] + BLOCKS[1 - s]
        kperm = np.concatenate([np.arange(bl * 128, (bl + 1) * 128) for bl in pord])
        # tile-contiguous: [(c2, k), 128, 1024] so each load is one block
        xT_in = np.ascontiguousarray(
            x[b][kperm].T.astype(bf16).reshape(8, 128, 2, 1024)
            .transpose(2, 0, 1, 3).reshape(2048, 1024))
        cosK_in, sinK_in = rope_pair(cos[kperm], sin[kperm], N)
        # v rotary tables in key-chunk order: [p, (ci, d)]; sign-folded sin
        cr = cos[kperm].reshape(16, 128, 32).transpose(1, 0, 2)
        sr = sin[kperm].reshape(16, 128, 32).transpose(1, 0, 2).copy()
        sr[:, :, 0:HALF] *= -1.0
        cosVs = np.ascontiguousarray(cr.reshape(128, 512)).astype(bf16)
        sinVs = np.ascontiguousarray(sr.reshape(128, 512)).astype(bf16)
        hs = slice(h * 512, (h + 1) * 512)
        wq_in = (w_qkv[:, 0:1024][:, hs] * SCALE).astype(bf16)
        wkv_in = np.concatenate(
            [w_qkv[:, 1024:2048][:, hs], w_qkv[:, 2048:3072][:, hs]], axis=1
        ).astype(bf16)
        # tile-contiguous: [k, 128, 1024]
        wkv_in = np.ascontiguousarray(wkv_in)
        # diagonal masks: own blocks get tri; partner diagonal chunk is
        # fully masked on even-q cores, fully visible on odd-q cores
        maskD_in = np.zeros((128, 256), np.float32)
        maskD_in[:, 0:128] = tri
        maskD_in[:, 128:256] = 0.0 if s == 0 else 1.0
        in_maps.append({
            "xT": xT_in,
            "wkv": wkv_in,
            "wq": wq_in,
            "wout": wout_p[:, h * 512:(h + 1) * 512].astype(bf16),
            "cosK": cosK_in, "sinK": sinK_in,
            "cosVs": cosVs, "sinVs": sinVs,
            "maskD": maskD_in.astype(bf16),
            "gw": g[h * 512:(h + 1) * 512].reshape(1, 512).astype(bf16),
        })
    return in_maps


def _maybe_install_trace_hook():
    import sys, types
    try:
        import antenv.axon_hooks  # noqa: F401
        return
    except ImportError:
        pass
    try:
        import trn_agent_boot.trn_boot as tb
        hook = tb._ntff_profile_via_ctypes('/opt/axon/libaxon_pjrt.so')
        mod = types.ModuleType('antenv.axon_hooks')
        mod.get_axon_ntff_profile_hook = lambda: hook
        mod.set_axon_ntff_profile_hook = lambda h: None
        sys.modules['antenv.axon_hooks'] = mod
    except Exception:
        pass


def kernel(x, mask, rotary_pos_emb, w_qkv, w_out, g):
    global LAST_EXEC_NS, LAST_RESULTS
    if "nc" not in _CACHE:
        _CACHE["nc"] = _build_program()
    nc = _CACHE["nc"]
    in_maps = _prep_inputs(x, rotary_pos_emb, w_qkv, w_out, g)
    kwargs = {}
    if TRACE:
        _maybe_install_trace_hook()
        kwargs = dict(trace=True, trace_cores=list(range(NCORES)))
    res = run_bass_kernel_spmd(nc, in_maps, core_ids=list(range(NCORES)), **kwargs)
    LAST_EXEC_NS = res.exec_time_ns
    LAST_RESULTS = res
    out = np.empty((B, N, DIM), np.float32)
    for b in range(B):
        for s in range(2):
            qperm = np.concatenate(
                [np.arange(bl * 128, (bl + 1) * 128) for bl in BLOCKS[s]])
            for h in range(2):
                core = b * 4 + s * 2 + h
                r = np.asarray(res.results[core]["out"], np.float32)
                out[b, qperm, h * 512:(h + 1) * 512] = r
    return out
